# revision 2
# baseline (speedup 1.0000x reference)
"""ARNet forward (teacher forcing) as a Trainium2 Bass kernel.

out[b, i] = sum_j w[j] * seq[b, i+j],  seq = concat(x, true_output[:, :63], axis=1)
          = (seq @ T)[b, i]            with T[k, i] = w[k-i] (Toeplitz, [127, 64])

Sharding: pure data parallel over the batch dim across 8 NeuronCores.

The kernel is DMA-bound (baseline trace: 99% DMA-active at ~330GB/s on
48.5MB/core of bf16 traffic). The rel-err budget (2e-2) admits int8 for the
seq wire format: per-batch-row symmetric quantization (scale = rowmax/127,
measured rel err 6.6e-3 incl. bf16 weights) halves input bytes vs bf16.
The PE can't consume int8, so DVE/Pool/ACT dequantize int8->bf16 in SBUF
(exact: |q|<=127 integers are exact in bf16); the row scale never touches
the device - it's folded into the host-side decode (out row b scales by
sc[b] since all 127 taps of row b share one scale).

Device-side work per core (125000 rows, cols padded to 125952 = 246*512):
  - Host builds int8 seqT blocks [30, 128, 4096] (+ [128, 3072] tail): row
    k (<127) = seq position k, column r = batch row. Each block is a fully
    contiguous 0.5MB DRAM region so the HWDGE fans it across all 16 SDMA
    engines in 4KB/partition packets.
  - Per block: one 0.5MB input DMA; dequant int8->bf16 split to balance
    engine busy (DVE chunks 0-3 @122.9G/s, Pool chunks 4-6 @~92G/s, ACT
    chunk 7 @153.6G/s - ACT also does all PSUM copies, ~2.1us/block each
    engine); 8 matmuls [127,64]x[127,512] -> PSUM [128,1024] tiles holding
    4 chunks each (even chunks at partitions 0-63, odd at 64-127 via
    matmul into PE columns 64-127); 2 half-tile PSUM->SBUF copies with
    fp32->bf16 cast on ACT (e-first order overlaps copy with matmul); one
    0.5MB output DMA.
  - Toeplitz lhsT is the only stationary; tail input prefetched at start.
"""

import sys

if "/opt/trn_rl_repo" not in sys.path:
    sys.path.insert(0, "/opt/trn_rl_repo")

import ml_dtypes
import numpy as np

import concourse.bacc as bacc
import concourse.mybir as mybir
import concourse.tile as tile
from concourse.bass_utils import run_bass_kernel_spmd

B = 1_000_000
N_LAGS = 64
NF = 64
SEQ = N_LAGS + NF - 1  # 127
N_CORES = 8
RPC = B // N_CORES  # 125000 rows per core

CHUNK = 512  # rows per matmul (= PSUM bank in fp32)
NCHUNKS = 246  # ceil(125000/512) rounded up to even (computed chunks)
CPB = 8  # chunks per full block
NBLK = 30  # full blocks; tail block has 6 chunks
TAILC = NCHUNKS - NBLK * CPB  # 6
BLKCOLS = CPB * CHUNK  # 4096
TAILCOLS = TAILC * CHUNK  # 3072

F32 = mybir.dt.float32
BF16 = mybir.dt.bfloat16
I8 = mybir.dt.int8
NP_BF16 = ml_dtypes.bfloat16

# column layout of the packed output: global pair J = 2*blk + t (tail J=60+t),
# out[h*64 + i, J*1024 + e*512 + s] = y[(4J + 2h + e)*512 + s, i]
NPAIRJ = NBLK * CPB // 4 + TAILC // 4 + (1 if TAILC % 4 else 0)  # 62
OUT_COLS = NPAIRJ * 2 * CHUNK  # 63488

# dequant engine split within a block (chunk granularity, balanced by
# engine rates: DVE 122.9 G/s, Pool ~92 G/s, ACT 153.6 G/s with ACT also
# covering the 2048 PSUM-copy cols per block)
DVE_CH = (0, 1, 2, 3)
POOL_CH = (4, 5, 6)
ACT_CH = (7,)

_cache = {}


def _build_nc():
    nc = bacc.Bacc("TRN2", target_bir_lowering=False, debug=False, num_devices=N_CORES)
    sqt = nc.dram_tensor("sqt", [NBLK, 128, BLKCOLS], I8, kind="ExternalInput")
    sqt_t = nc.dram_tensor("sqt_t", [128, TAILCOLS], I8, kind="ExternalInput")
    tpl = nc.dram_tensor("tpl", [128, NF], BF16, kind="ExternalInput")
    out = nc.dram_tensor("out", [128, OUT_COLS], BF16, kind="ExternalOutput")

    with tile.TileContext(nc) as tc:
        with (
            tc.tile_pool(name="consts", bufs=1) as consts,
            tc.tile_pool(name="sqin", bufs=12) as spool,
            tc.tile_pool(name="conv", bufs=6) as cpool,
            tc.tile_pool(name="oout", bufs=10) as opool,
            tc.tile_pool(name="psO", bufs=4, space="PSUM") as psO,
        ):
            tpl_sb = consts.tile([128, NF], BF16)
            nc.sync.dma_start(tpl_sb[:], tpl.ap())
            # tail block's input, prefetched at start so the final compute
            # chain never waits on the last-arriving DMA
            s_tail = consts.tile([128, TAILCOLS], I8)
            nc.sync.dma_start(s_tail[:], sqt_t.ap())

            for b in range(NBLK + 1):
                is_tail = b == NBLK
                nch = TAILC if is_tail else CPB
                if is_tail:
                    s8 = s_tail
                else:
                    s8 = spool.tile([128, BLKCOLS], I8, tag="sqin")
                    nc.sync.dma_start(s8[:], sqt.ap()[b])
                sb = cpool.tile([128, nch * CHUNK], BF16, tag="conv")
                # dequant int8 -> bf16 (exact for small ints), split by engine
                for eng, chs in (
                    (nc.vector, DVE_CH),
                    (nc.gpsimd, POOL_CH),
                    (nc.scalar, ACT_CH),
                ):
                    chs = [c for c in chs if c < nch]
                    if not chs:
                        continue
                    lo, hi = chs[0] * CHUNK, (chs[-1] + 1) * CHUNK
                    if eng is nc.scalar:
                        nc.scalar.copy(sb[:, lo:hi], s8[:, lo:hi])
                    else:
                        eng.tensor_copy(sb[:, lo:hi], s8[:, lo:hi])
                o_t = opool.tile([128, (CPB // 2) * CHUNK], BF16, tag="oout")
                for t in range((nch + 3) // 4):
                    ps = psO.tile([128, 2 * CHUNK], F32, tag="psO")
                    # e-first order so the e=0 half-copy fires while e=1 runs
                    for e in range(2):
                        for h in range(2):
                            c = 4 * t + 2 * h + e
                            if c < nch:
                                nc.tensor.matmul(
                                    ps[h * 64 : h * 64 + 64, e * CHUNK : (e + 1) * CHUNK],
                                    tpl_sb[0:SEQ, :],
                                    sb[0:SEQ, c * CHUNK : (c + 1) * CHUNK],
                                    start=True,
                                    stop=True,
                                )
                        psrc = ps[:, e * CHUNK : (e + 1) * CHUNK]
                        dst = o_t[:, t * 1024 + e * CHUNK : t * 1024 + (e + 1) * CHUNK]
                        nc.scalar.copy(dst, psrc)
                ocols = ((nch + 3) // 4) * 1024  # cols actually written
                nc.scalar.dma_start(
                    out.ap()[:, b * 2048 : b * 2048 + ocols], o_t[:, 0:ocols]
                )
    nc.compile()
    return nc


def _get_nc():
    if "nc" not in _cache:
        _cache["nc"] = _build_nc()
    return _cache["nc"]


def _quantize(x, true_output):
    """Per-row symmetric int8: q = round(seq/sc), sc = rowmax/127."""
    seq = np.concatenate(
        [np.asarray(x, np.float32), np.asarray(true_output, np.float32)[:, : NF - 1]],
        axis=1,
    )  # [B, 127]
    sc = np.maximum(np.abs(seq).max(axis=1), 1e-30) / 127.0  # [B]
    q = np.rint(seq / sc[:, None])
    np.clip(q, -127, 127, out=q)
    return q.astype(np.int8), sc.astype(np.float32)


def _prepare_in_maps(x, true_output, w):
    q, sc = _quantize(x, true_output)
    w = np.asarray(w, dtype=np.float32).reshape(N_LAGS)

    tpl = np.zeros((128, NF), np.float32)
    for i in range(NF):
        tpl[i : i + N_LAGS, i] = w
    tpl = tpl.astype(NP_BF16)

    cols = NBLK * BLKCOLS + TAILCOLS  # 125952
    flat = np.zeros((N_CORES, SEQ, cols), np.int8)
    for c in range(N_CORES):
        rows = slice(c * RPC, (c + 1) * RPC)
        flat[c, :, :RPC] = q[rows].T
    sqt = np.zeros((N_CORES, NBLK, 128, BLKCOLS), np.int8)
    sqt[:, :, :SEQ, :] = (
        flat[:, :, : NBLK * BLKCOLS]
        .reshape(N_CORES, SEQ, NBLK, BLKCOLS)
        .swapaxes(1, 2)
    )
    sqt_t = np.zeros((N_CORES, 128, TAILCOLS), np.int8)
    sqt_t[:, :SEQ, :] = flat[:, :, NBLK * BLKCOLS :]

    in_maps = [
        {"sqt": sqt[c], "sqt_t": sqt_t[c], "tpl": tpl} for c in range(N_CORES)
    ]
    return in_maps, sc


def _decode_out(results, sc):
    outs = []
    for c, r in enumerate(results):
        oh = np.asarray(r["out"]).reshape(2, 64, NPAIRJ, 2, CHUNK)  # h,i,J,e,s
        full = oh.transpose(2, 0, 3, 4, 1).reshape(NPAIRJ * 4 * CHUNK, NF)
        rows = slice(c * RPC, (c + 1) * RPC)
        outs.append(full[:RPC].astype(np.float32) * sc[rows, None])
    return np.concatenate(outs, axis=0)


def kernel(x, true_output, w):
    nc = _get_nc()
    in_maps, sc = _prepare_in_maps(x, true_output, w)
    res = run_bass_kernel_spmd(nc, in_maps, core_ids=list(range(N_CORES)))
    return _decode_out(res.results, sc)


def run_traced(x, true_output, w, tmpdir=None):
    """Like kernel() but captures an NTFF profile; returns (out, BassKernelResults)."""
    import types

    import antenv
    import concourse.bass_utils as bass_utils

    if "antenv.axon_hooks" not in sys.modules:
        hooks_mod = types.ModuleType("antenv.axon_hooks")
        _hook = [None]
        hooks_mod.set_axon_ntff_profile_hook = lambda h: _hook.__setitem__(0, h)
        hooks_mod.get_axon_ntff_profile_hook = lambda: _hook[0]
        sys.modules["antenv.axon_hooks"] = hooks_mod
        antenv.axon_hooks = hooks_mod
        from trn_agent_boot.trn_boot import _ntff_profile_via_ctypes

        hooks_mod.set_axon_ntff_profile_hook(
            _ntff_profile_via_ctypes("/opt/axon/libaxon_pjrt.so")
        )
    bass_utils.upload_artifacts = lambda d: d  # no S3 in this container

    if tmpdir is not None:
        import shutil

        shutil.rmtree(tmpdir, ignore_errors=True)

    nc = _get_nc()
    in_maps, sc = _prepare_in_maps(x, true_output, w)
    res = run_bass_kernel_spmd(
        nc, in_maps, core_ids=list(range(N_CORES)), trace=True, tmpdir=tmpdir
    )
    return _decode_out(res.results, sc), res


# revision 3
# speedup vs baseline: 1.9270x; 1.9270x over previous
"""ARNet forward (teacher forcing) as a Trainium2 Bass kernel.

out[b, i] = sum_j w[j] * seq[b, i+j],  seq = concat(x, true_output[:, :63], axis=1)
          = (seq @ T)[b, i]            with T[k, i] = w[k-i] (Toeplitz, [127, 64])

Sharding: pure data parallel over the batch dim across 8 NeuronCores.

The kernel is DMA-bound (bf16 baseline trace: 99% DMA-active at ~330GB/s on
48.5MB/core). The rel-err budget (2e-2) admits int8 on BOTH wires (measured
1.43e-2 end to end):
  - input: per-batch-row symmetric quantization of seq (sc = rowmax/127);
    the PE can't consume int8 so DVE casts int8->bf16 in SBUF (exact for
    small ints; measured 229G elem/s - do NOT use the Pool engine, its
    software casts run at 27G/s and its SBUF thrashing slows DVE/PE ~3x).
    The row scale never touches the device: out row b scales by sc[b]
    (all 127 taps of row b share one scale), folded into host decode.
  - output: global scale gamma, FOLDED INTO THE TOEPLITZ (tpl = bf16(w/g))
    so PSUM already holds out/(g*sc) and the ACT PSUM->SBUF copy is a plain
    saturating fp32->int8 cast (round-nearest-even, +-127/-128; saturation
    verified on HW). gamma is estimated from a host-side 2% subsample conv
    (x1.08), so rare outliers just saturate (~1e-6 of elements).

Device-side work per core (125000 rows, cols padded to 125952 = 246*512):
  - Host builds int8 seqT blocks [30, 128, 4096] (+ [128, 3072] tail): row
    k (<127) = seq position k, column r = batch row. Each block is a fully
    contiguous 0.5MB DRAM region so the HWDGE fans it across all 16 SDMA
    engines in 4KB/partition packets.
  - Per block: one 0.5MB input DMA; 2 DVE casts [128,2048] int8->bf16;
    8 matmuls [127,64]x[127,512] -> PSUM [128,1024] tiles holding 4 chunks
    (even chunks at partitions 0-63, odd at 64-127 via matmul into PE
    columns 64-127); 4 ACT copies [128,512] PSUM->int8 SBUF (e-first order
    overlaps copy with matmul). Output tiles pair TWO blocks ([128,4096]
    int8 = 4KB/partition) so output DMA packets stay at the 4KB sweet spot.
  - Toeplitz lhsT is the only stationary; tail input prefetched at start.
"""

import sys

if "/opt/trn_rl_repo" not in sys.path:
    sys.path.insert(0, "/opt/trn_rl_repo")

import ml_dtypes
import numpy as np

import concourse.bacc as bacc
import concourse.mybir as mybir
import concourse.tile as tile
from concourse.bass_utils import run_bass_kernel_spmd

B = 1_000_000
N_LAGS = 64
NF = 64
SEQ = N_LAGS + NF - 1  # 127
N_CORES = 8
RPC = B // N_CORES  # 125000 rows per core

CHUNK = 512  # rows per matmul (= PSUM bank in fp32)
NCHUNKS = 246  # ceil(125000/512) rounded up to even (computed chunks)
CPB = 8  # chunks per full block
NBLK = 30  # full blocks; tail block has 6 chunks
TAILC = NCHUNKS - NBLK * CPB  # 6
BLKCOLS = CPB * CHUNK  # 4096
TAILCOLS = TAILC * CHUNK  # 3072

F32 = mybir.dt.float32
BF16 = mybir.dt.bfloat16
I8 = mybir.dt.int8
NP_BF16 = ml_dtypes.bfloat16

# column layout of the packed output: global pair J = 2*blk + t (tail J=60+t),
# out[h*64 + i, J*1024 + e*512 + s] = y[(4J + 2h + e)*512 + s, i] / (gamma*sc)
NPAIRJ = NBLK * CPB // 4 + TAILC // 4 + (1 if TAILC % 4 else 0)  # 62
OUT_COLS = NPAIRJ * 2 * CHUNK  # 63488

_cache = {}


def _build_nc():
    nc = bacc.Bacc("TRN2", target_bir_lowering=False, debug=False, num_devices=N_CORES)
    sqt = nc.dram_tensor("sqt", [NBLK, 128, BLKCOLS], I8, kind="ExternalInput")
    sqt_t = nc.dram_tensor("sqt_t", [128, TAILCOLS], I8, kind="ExternalInput")
    tpl = nc.dram_tensor("tpl", [128, NF], BF16, kind="ExternalInput")
    out = nc.dram_tensor("out", [128, OUT_COLS], I8, kind="ExternalOutput")

    with tile.TileContext(nc) as tc:
        with (
            tc.tile_pool(name="consts", bufs=1) as consts,
            tc.tile_pool(name="sqin", bufs=12) as spool,
            tc.tile_pool(name="conv", bufs=6) as cpool,
            tc.tile_pool(name="oout", bufs=6) as opool,
            tc.tile_pool(name="psO", bufs=4, space="PSUM") as psO,
        ):
            tpl_sb = consts.tile([128, NF], BF16)
            nc.sync.dma_start(tpl_sb[:], tpl.ap())
            # tail block's input, prefetched at start so the final compute
            # chain never waits on the last-arriving DMA
            s_tail = consts.tile([128, TAILCOLS], I8)
            nc.sync.dma_start(s_tail[:], sqt_t.ap())

            o_t = None
            for b in range(NBLK + 1):
                is_tail = b == NBLK
                nch = TAILC if is_tail else CPB
                if is_tail:
                    s8 = s_tail
                else:
                    s8 = spool.tile([128, BLKCOLS], I8, tag="sqin")
                    nc.sync.dma_start(s8[:], sqt.ap()[b])
                sb = cpool.tile([128, nch * CHUNK], BF16, tag="conv")
                # dequant int8 -> bf16 on DVE only (229G elem/s; Pool is poison)
                half = (nch * CHUNK) // 2
                nc.vector.tensor_copy(sb[:, 0:half], s8[:, 0:half])
                nc.vector.tensor_copy(sb[:, half : nch * CHUNK], s8[:, half : nch * CHUNK])
                # output tile spans a PAIR of blocks (4KB/partition DMA grain)
                if is_tail:
                    o_t = opool.tile([128, 2 * 1024], I8, tag="oout")
                    obase = 0
                elif b % 2 == 0:
                    o_t = opool.tile([128, 2 * (CPB // 2) * CHUNK], I8, tag="oout")
                    obase = 0
                else:
                    obase = (CPB // 2) * CHUNK  # second half of the pair tile
                for t in range((nch + 3) // 4):
                    ps = psO.tile([128, 2 * CHUNK], F32, tag="psO")
                    # e-first order so the e=0 half-copy fires while e=1 runs
                    for e in range(2):
                        for h in range(2):
                            c = 4 * t + 2 * h + e
                            if c < nch:
                                nc.tensor.matmul(
                                    ps[h * 64 : h * 64 + 64, e * CHUNK : (e + 1) * CHUNK],
                                    tpl_sb[0:SEQ, :],
                                    sb[0:SEQ, c * CHUNK : (c + 1) * CHUNK],
                                    start=True,
                                    stop=True,
                                )
                        psrc = ps[:, e * CHUNK : (e + 1) * CHUNK]
                        dst = o_t[
                            :, obase + t * 1024 + e * CHUNK : obase + t * 1024 + (e + 1) * CHUNK
                        ]
                        nc.scalar.copy(dst, psrc)  # fp32 -> int8 saturating cast
                ocols = ((nch + 3) // 4) * 1024  # cols written by this block
                if is_tail:
                    nc.scalar.dma_start(
                        out.ap()[:, b * 2048 : b * 2048 + ocols], o_t[:, 0:ocols]
                    )
                elif b % 2 == 1:
                    nc.scalar.dma_start(
                        out.ap()[:, (b - 1) * 2048 : (b + 1) * 2048], o_t[:]
                    )
    nc.compile()
    return nc


def _get_nc():
    if "nc" not in _cache:
        _cache["nc"] = _build_nc()
    return _cache["nc"]


def _toeplitz(wv):
    tpl = np.zeros((128, NF), np.float32)
    for i in range(NF):
        tpl[i : i + N_LAGS, i] = wv
    return tpl


def _prepare_in_maps(x, true_output, w):
    seq = np.concatenate(
        [np.asarray(x, np.float32), np.asarray(true_output, np.float32)[:, : NF - 1]],
        axis=1,
    )  # [B, 127]
    sc = np.maximum(np.abs(seq).max(axis=1), 1e-30) / 127.0  # [B]
    q = np.rint(seq / sc[:, None])
    np.clip(q, -127, 127, out=q)
    q = q.astype(np.int8)
    w = np.asarray(w, dtype=np.float32).reshape(N_LAGS)

    # estimate the output scale gamma from a 2% subsample of the quantized
    # conv (outliers beyond it just saturate the int8 cast on device)
    rng = np.random.default_rng(12345)
    idx = rng.choice(B, B // 50, replace=False)
    tpl_plain = _toeplitz(w.astype(NP_BF16).astype(np.float32))[:SEQ]
    psum_sub = q[idx].astype(np.float32) @ tpl_plain
    gamma = np.float32(np.abs(psum_sub).max() * 1.08 / 127.0)

    tpl = _toeplitz(w / gamma).astype(NP_BF16)

    cols = NBLK * BLKCOLS + TAILCOLS  # 125952
    flat = np.zeros((N_CORES, SEQ, cols), np.int8)
    for c in range(N_CORES):
        rows = slice(c * RPC, (c + 1) * RPC)
        flat[c, :, :RPC] = q[rows].T
    sqt = np.zeros((N_CORES, NBLK, 128, BLKCOLS), np.int8)
    sqt[:, :, :SEQ, :] = (
        flat[:, :, : NBLK * BLKCOLS]
        .reshape(N_CORES, SEQ, NBLK, BLKCOLS)
        .swapaxes(1, 2)
    )
    sqt_t = np.zeros((N_CORES, 128, TAILCOLS), np.int8)
    sqt_t[:, :SEQ, :] = flat[:, :, NBLK * BLKCOLS :]

    in_maps = [
        {"sqt": sqt[c], "sqt_t": sqt_t[c], "tpl": tpl} for c in range(N_CORES)
    ]
    return in_maps, sc, gamma


def _decode_out(results, sc, gamma):
    outs = []
    for c, r in enumerate(results):
        oh = np.asarray(r["out"]).reshape(2, 64, NPAIRJ, 2, CHUNK)  # h,i,J,e,s
        full = oh.transpose(2, 0, 3, 4, 1).reshape(NPAIRJ * 4 * CHUNK, NF)
        rows = slice(c * RPC, (c + 1) * RPC)
        outs.append(full[:RPC].astype(np.float32) * (gamma * sc[rows, None]))
    return np.concatenate(outs, axis=0)


def kernel(x, true_output, w):
    nc = _get_nc()
    in_maps, sc, gamma = _prepare_in_maps(x, true_output, w)
    res = run_bass_kernel_spmd(nc, in_maps, core_ids=list(range(N_CORES)))
    return _decode_out(res.results, sc, gamma)


def run_traced(x, true_output, w, tmpdir=None):
    """Like kernel() but captures an NTFF profile; returns (out, BassKernelResults)."""
    import types

    import antenv
    import concourse.bass_utils as bass_utils

    if "antenv.axon_hooks" not in sys.modules:
        hooks_mod = types.ModuleType("antenv.axon_hooks")
        _hook = [None]
        hooks_mod.set_axon_ntff_profile_hook = lambda h: _hook.__setitem__(0, h)
        hooks_mod.get_axon_ntff_profile_hook = lambda: _hook[0]
        sys.modules["antenv.axon_hooks"] = hooks_mod
        antenv.axon_hooks = hooks_mod
        from trn_agent_boot.trn_boot import _ntff_profile_via_ctypes

        hooks_mod.set_axon_ntff_profile_hook(
            _ntff_profile_via_ctypes("/opt/axon/libaxon_pjrt.so")
        )
    bass_utils.upload_artifacts = lambda d: d  # no S3 in this container

    if tmpdir is not None:
        import shutil

        shutil.rmtree(tmpdir, ignore_errors=True)

    nc = _get_nc()
    in_maps, sc, gamma = _prepare_in_maps(x, true_output, w)
    res = run_bass_kernel_spmd(
        nc, in_maps, core_ids=list(range(N_CORES)), trace=True, tmpdir=tmpdir
    )
    return _decode_out(res.results, sc, gamma), res


# revision 4
# speedup vs baseline: 2.0083x; 1.0422x over previous
"""ARNet forward (teacher forcing) as a Trainium2 Bass kernel.

out[b, i] = sum_j w[j] * seq[b, i+j],  seq = concat(x, true_output[:, :63], axis=1)
          = (seq @ T)[b, i]            with T[k, i] = w[k-i] (Toeplitz, [127, 64])

Sharding: pure data parallel over the batch dim across 8 NeuronCores.

The kernel is DMA-bound (bf16 baseline trace: 99% DMA-active at ~330GB/s on
48.5MB/core). The rel-err budget (2e-2) admits int8 on BOTH wires (measured
1.43e-2 end to end):
  - input: per-batch-row symmetric quantization of seq (sc = rowmax/127);
    the PE can't consume int8 so DVE casts int8->bf16 in SBUF (exact for
    small ints; measured 229G elem/s - do NOT use the Pool engine, its
    software casts run at 27G/s and its SBUF thrashing slows DVE/PE ~3x).
    The row scale never touches the device: out row b scales by sc[b]
    (all 127 taps of row b share one scale), folded into host decode.
  - output: global scale gamma, FOLDED INTO THE TOEPLITZ (tpl = bf16(w/g))
    so PSUM already holds out/(g*sc) and the ACT PSUM->SBUF copy is a plain
    saturating fp32->int8 cast (round-nearest-even, +-127/-128; saturation
    verified on HW). gamma is estimated from a host-side 2% subsample conv
    (x1.08), so rare outliers just saturate (~1e-6 of elements).

Device-side work per core (125000 rows, cols padded to 125952 = 246*512):
  - Host builds int8 seqT blocks [30, 128, 4096] (+ [128, 3072] tail): row
    k (<127) = seq position k, column r = batch row. Each block is a fully
    contiguous 0.5MB DRAM region so the HWDGE fans it across all 16 SDMA
    engines in 4KB/partition packets.
  - Per block: one 0.5MB input DMA; 2 DVE casts [128,2048] int8->bf16;
    8 matmuls [127,64]x[127,512] -> PSUM [128,1024] tiles holding 4 chunks
    (even chunks at partitions 0-63, odd at 64-127 via matmul into PE
    columns 64-127); 4 ACT copies [128,512] PSUM->int8 SBUF (e-first order
    overlaps copy with matmul). Output tiles pair TWO blocks ([128,4096]
    int8 = 4KB/partition) so output DMA packets stay at the 4KB sweet spot.
  - Toeplitz lhsT is the only stationary; tail input prefetched at start.
"""

import sys

if "/opt/trn_rl_repo" not in sys.path:
    sys.path.insert(0, "/opt/trn_rl_repo")

import ml_dtypes
import numpy as np

import concourse.bacc as bacc
import concourse.mybir as mybir
import concourse.tile as tile
from concourse.bass_utils import run_bass_kernel_spmd

B = 1_000_000
N_LAGS = 64
NF = 64
SEQ = N_LAGS + NF - 1  # 127
N_CORES = 8
RPC = B // N_CORES  # 125000 rows per core

CHUNK = 512  # rows per matmul (= PSUM bank in fp32)
NCHUNKS = 246  # ceil(125000/512) rounded up to even (computed chunks)
CPB = 8  # chunks per full block
NBLK = 30  # full blocks; tail block has 6 chunks
TAILC = NCHUNKS - NBLK * CPB  # 6
BLKCOLS = CPB * CHUNK  # 4096
TAILCOLS = TAILC * CHUNK  # 3072

F32 = mybir.dt.float32
BF16 = mybir.dt.bfloat16
I8 = mybir.dt.int8
NP_BF16 = ml_dtypes.bfloat16

# column layout of the packed output: global pair J = 2*blk + t (tail J=60+t),
# out[h*64 + i, J*1024 + e*512 + s] = y[(4J + 2h + e)*512 + s, i] / (gamma*sc)
NPAIRJ = NBLK * CPB // 4 + TAILC // 4 + (1 if TAILC % 4 else 0)  # 62
OUT_COLS = NPAIRJ * 2 * CHUNK  # 63488

_cache = {}


def _build_nc():
    nc = bacc.Bacc("TRN2", target_bir_lowering=False, debug=False, num_devices=N_CORES)
    sqt = nc.dram_tensor("sqt", [NBLK, 128, BLKCOLS], I8, kind="ExternalInput")
    sqt_t = nc.dram_tensor("sqt_t", [128, TAILCOLS], I8, kind="ExternalInput")
    tpl = nc.dram_tensor("tpl", [128, NF], BF16, kind="ExternalInput")
    out = nc.dram_tensor("out", [128, OUT_COLS], I8, kind="ExternalOutput")

    with tile.TileContext(nc) as tc:
        with (
            tc.tile_pool(name="consts", bufs=1) as consts,
            tc.tile_pool(name="sqin", bufs=12) as spool,
            tc.tile_pool(name="conv", bufs=6) as cpool,
            tc.tile_pool(name="oout", bufs=6) as opool,
            tc.tile_pool(name="psO", bufs=4, space="PSUM") as psO,
        ):
            tpl_sb = consts.tile([128, NF], BF16)
            nc.sync.dma_start(tpl_sb[:], tpl.ap())
            # tail block's input, prefetched at start so the final compute
            # chain never waits on the last-arriving DMA
            s_tail = consts.tile([128, TAILCOLS], I8)
            nc.sync.dma_start(s_tail[:], sqt_t.ap())

            o_t = None
            for b in range(NBLK + 1):
                is_tail = b == NBLK
                nch = TAILC if is_tail else CPB
                if is_tail:
                    s8 = s_tail
                else:
                    s8 = spool.tile([128, BLKCOLS], I8, tag="sqin")
                    nc.sync.dma_start(s8[:], sqt.ap()[b])
                sb = cpool.tile([128, nch * CHUNK], BF16, tag="conv")
                # dequant int8 -> bf16 on DVE only (229G elem/s; Pool is poison)
                half = (nch * CHUNK) // 2
                nc.vector.tensor_copy(sb[:, 0:half], s8[:, 0:half])
                nc.vector.tensor_copy(sb[:, half : nch * CHUNK], s8[:, half : nch * CHUNK])
                # output tile spans a PAIR of blocks (4KB/partition DMA grain)
                if is_tail:
                    o_t = opool.tile([128, 2 * 1024], I8, tag="oout")
                    obase = 0
                elif b % 2 == 0:
                    o_t = opool.tile([128, 2 * (CPB // 2) * CHUNK], I8, tag="oout")
                    obase = 0
                else:
                    obase = (CPB // 2) * CHUNK  # second half of the pair tile
                for t in range((nch + 3) // 4):
                    ps = psO.tile([128, 2 * CHUNK], F32, tag="psO")
                    for e in range(2):
                        for h in range(2):
                            c = 4 * t + 2 * h + e
                            if c < nch:
                                nc.tensor.matmul(
                                    ps[h * 64 : h * 64 + 64, e * CHUNK : (e + 1) * CHUNK],
                                    tpl_sb[0:SEQ, :],
                                    sb[0:SEQ, c * CHUNK : (c + 1) * CHUNK],
                                    start=True,
                                    stop=True,
                                )
                    # whole-tile fp32 -> int8 saturating cast: 1024-col grain
                    # amortizes ACT's ~330ns per-instruction access overhead
                    dst = o_t[:, obase + t * 1024 : obase + (t + 1) * 1024]
                    nc.scalar.copy(dst, ps[:])
                ocols = ((nch + 3) // 4) * 1024  # cols written by this block
                # descriptor issue on the otherwise-idle Pool sequencer keeps
                # ACT 100% on PSUM copies (Q7 compute ucode stays unused)
                if is_tail:
                    nc.gpsimd.dma_start(
                        out.ap()[:, b * 2048 : b * 2048 + ocols], o_t[:, 0:ocols]
                    )
                elif b % 2 == 1:
                    nc.gpsimd.dma_start(
                        out.ap()[:, (b - 1) * 2048 : (b + 1) * 2048], o_t[:]
                    )
    nc.compile()
    return nc


def _get_nc():
    if "nc" not in _cache:
        _cache["nc"] = _build_nc()
    return _cache["nc"]


def _toeplitz(wv):
    tpl = np.zeros((128, NF), np.float32)
    for i in range(NF):
        tpl[i : i + N_LAGS, i] = wv
    return tpl


def _prepare_in_maps(x, true_output, w):
    seq = np.concatenate(
        [np.asarray(x, np.float32), np.asarray(true_output, np.float32)[:, : NF - 1]],
        axis=1,
    )  # [B, 127]
    sc = np.maximum(np.abs(seq).max(axis=1), 1e-30) / 127.0  # [B]
    q = np.rint(seq / sc[:, None])
    np.clip(q, -127, 127, out=q)
    q = q.astype(np.int8)
    w = np.asarray(w, dtype=np.float32).reshape(N_LAGS)

    # estimate the output scale gamma from a 2% subsample of the quantized
    # conv (outliers beyond it just saturate the int8 cast on device)
    rng = np.random.default_rng(12345)
    idx = rng.choice(B, B // 50, replace=False)
    tpl_plain = _toeplitz(w.astype(NP_BF16).astype(np.float32))[:SEQ]
    psum_sub = q[idx].astype(np.float32) @ tpl_plain
    gamma = np.float32(np.abs(psum_sub).max() * 1.08 / 127.0)

    tpl = _toeplitz(w / gamma).astype(NP_BF16)

    cols = NBLK * BLKCOLS + TAILCOLS  # 125952
    flat = np.zeros((N_CORES, SEQ, cols), np.int8)
    for c in range(N_CORES):
        rows = slice(c * RPC, (c + 1) * RPC)
        flat[c, :, :RPC] = q[rows].T
    sqt = np.zeros((N_CORES, NBLK, 128, BLKCOLS), np.int8)
    sqt[:, :, :SEQ, :] = (
        flat[:, :, : NBLK * BLKCOLS]
        .reshape(N_CORES, SEQ, NBLK, BLKCOLS)
        .swapaxes(1, 2)
    )
    sqt_t = np.zeros((N_CORES, 128, TAILCOLS), np.int8)
    sqt_t[:, :SEQ, :] = flat[:, :, NBLK * BLKCOLS :]

    in_maps = [
        {"sqt": sqt[c], "sqt_t": sqt_t[c], "tpl": tpl} for c in range(N_CORES)
    ]
    return in_maps, sc, gamma


def _decode_out(results, sc, gamma):
    outs = []
    for c, r in enumerate(results):
        oh = np.asarray(r["out"]).reshape(2, 64, NPAIRJ, 2, CHUNK)  # h,i,J,e,s
        full = oh.transpose(2, 0, 3, 4, 1).reshape(NPAIRJ * 4 * CHUNK, NF)
        rows = slice(c * RPC, (c + 1) * RPC)
        outs.append(full[:RPC].astype(np.float32) * (gamma * sc[rows, None]))
    return np.concatenate(outs, axis=0)


def kernel(x, true_output, w):
    nc = _get_nc()
    in_maps, sc, gamma = _prepare_in_maps(x, true_output, w)
    res = run_bass_kernel_spmd(nc, in_maps, core_ids=list(range(N_CORES)))
    return _decode_out(res.results, sc, gamma)


def run_traced(x, true_output, w, tmpdir=None):
    """Like kernel() but captures an NTFF profile; returns (out, BassKernelResults)."""
    import types

    import antenv
    import concourse.bass_utils as bass_utils

    if "antenv.axon_hooks" not in sys.modules:
        hooks_mod = types.ModuleType("antenv.axon_hooks")
        _hook = [None]
        hooks_mod.set_axon_ntff_profile_hook = lambda h: _hook.__setitem__(0, h)
        hooks_mod.get_axon_ntff_profile_hook = lambda: _hook[0]
        sys.modules["antenv.axon_hooks"] = hooks_mod
        antenv.axon_hooks = hooks_mod
        from trn_agent_boot.trn_boot import _ntff_profile_via_ctypes

        hooks_mod.set_axon_ntff_profile_hook(
            _ntff_profile_via_ctypes("/opt/axon/libaxon_pjrt.so")
        )
    bass_utils.upload_artifacts = lambda d: d  # no S3 in this container

    if tmpdir is not None:
        import shutil

        shutil.rmtree(tmpdir, ignore_errors=True)

    nc = _get_nc()
    in_maps, sc, gamma = _prepare_in_maps(x, true_output, w)
    res = run_bass_kernel_spmd(
        nc, in_maps, core_ids=list(range(N_CORES)), trace=True, tmpdir=tmpdir
    )
    return _decode_out(res.results, sc, gamma), res
